# revision 48
# baseline (speedup 1.0000x reference)
"""Trainium2 Bass kernel for nn_EncodingLayer (gnn message passing).

Math (matches reference.py):
  x = points^T [B,N,C]; xpad = cat([x, -1]); near = xpad[:, near_idx]      (gather)
  feat = cat([x bcast over k, near])  -> 3x (1x1 conv + BN(train stats) + relu)
  new_feature = max over 6 neighbors; out_feature = max over channels.

Strategy: shard N over 8 cores.  Per core, the gather is done from an SBUF
table of this core's *unique* neighbor vertices (host computes the unique set
and remaps indices to local int16; table width < 32768 so GPSIMD ap_gather
applies, full fp32).  Layer l BN stats (sum, sumsq per channel) are
accumulated on-chip and combined with one 8-core AllReduce per layer; the
BN+relu collapses into the scalar-engine activation (per-partition scale and
bias).  Layer-1 activations are recomputed in phase C instead of stored so
everything stays in SBUF.  Neighbor-max commutes with the (monotone) final
affine+relu, so it runs on pre-BN y2 chunks straight out of PSUM.
"""

import numpy as np

B, C, N, KN = 2, 64, 40000, 6
NCORES = 8
NS = N // NCORES          # vertices per core
F = NS * KN               # free-size of per-core row axis (k-inner)
U_CAP = 16384             # compile-time gather-table width (>= per-core unique count)
CNT = float(B * N * KN)   # BN normalization count
EPS = 1e-5
GC = 3360                 # gather chunk (columns)
MC = 480                  # matmul/psum chunk (80 vertices x 6)


def _gather_chunks(f_tot, gc):
    out = []
    off = 0
    col = 0
    while off < f_tot:
        sz = min(gc, f_tot - off)
        assert sz % 16 == 0 and sz % KN == 0
        out.append((off, sz, col))
        col += sz // 16
        off += sz
    return out


def _mm_chunks(f_tot, gc, mc):
    out = []
    ci = 0
    for g_off, g_sz, _ in _gather_chunks(f_tot, gc):
        o = 0
        while o < g_sz:
            w = min(mc, g_sz - o)
            assert w % KN == 0
            out.append((g_off + o, w, ci))
            ci += 1
            o += w
    return out


def _build(u_cap=U_CAP, f_tot=F, ns=NS, idx_cols=None, n_slots=None, cnt=CNT, collectives=True, n_dev=NCORES):
    import concourse.bass as bass
    import concourse.mybir as mybir
    import concourse.bacc as bacc
    import concourse.tile as tile
    import concourse.bass_isa as bass_isa

    gcs = _gather_chunks(f_tot, GC)
    mcs = _mm_chunks(f_tot, GC, MC)
    if idx_cols is None:
        idx_cols = gcs[-1][2] + gcs[-1][1] // 16
    if n_slots is None:
        n_slots = len(mcs)

    dt = mybir.dt
    AF = mybir.ActivationFunctionType
    OP = mybir.AluOpType

    nc = bacc.Bacc("TRN2", target_bir_lowering=False, debug=False,
                   num_devices=n_dev)

    # ---- kernel I/O ----
    xg_d = nc.dram_tensor("xg", [128, u_cap], dt.float32, kind="ExternalInput")
    lidx_d = nc.dram_tensor("lidx", [128, idx_cols], dt.int16, kind="ExternalInput")
    l0a_d = nc.dram_tensor("l0a", [128, 128], dt.float32, kind="ExternalInput")
    l0b_d = nc.dram_tensor("l0b", [128, 128], dt.float32, kind="ExternalInput")
    l1_d = nc.dram_tensor("l1", [128, 128], dt.float32, kind="ExternalInput")
    l2_d = nc.dram_tensor("l2", [128, 128], dt.float32, kind="ExternalInput")
    cst_d = nc.dram_tensor("cst", [128, 12], dt.float32, kind="ExternalInput")
    newf_d = nc.dram_tensor("newf", [128, 2, ns], dt.float32, kind="ExternalOutput")
    outf_d = nc.dram_tensor("outf", [2, ns], dt.float32, kind="ExternalOutput")
    dbg_d = nc.dram_tensor("dbg", [128, 64], dt.float32, kind="ExternalOutput")

    rg = [list(range(NCORES))]

    with tile.TileContext(nc) as tc:
        with (
            tc.tile_pool(name="bigp", bufs=1) as bigp,
            tc.tile_pool(name="resid", bufs=1) as resid,
            tc.tile_pool(name="gpool", bufs=1) as gpool,
            tc.tile_pool(name="scr", bufs=1) as scr,
            tc.tile_pool(name="x2p", bufs=2) as x2p,
            tc.tile_pool(name="tiny", bufs=1) as tiny,
            tc.tile_pool(name="prp", bufs=1) as prp,
            tc.tile_pool(name="psum", bufs=2, space="PSUM") as psp,
            tc.tile_pool(name="dram", bufs=6, space="DRAM") as drp,
        ):
            # ---- resident tiles ----
            xg = bigp.tile([128, u_cap], dt.float32, tag="big")
            y0 = resid.tile([128, f_tot], dt.float32)
            l0a = resid.tile([128, 128], dt.float32)
            l0b = resid.tile([128, 128], dt.float32)
            l1w = resid.tile([128, 128], dt.float32)
            l2w = resid.tile([128, 128], dt.float32)
            cst = resid.tile([128, 12], dt.float32)

            nc.sync.dma_start(xg[:], xg_d.ap())
            nc.sync.dma_start(l0a[:], l0a_d.ap())
            nc.sync.dma_start(l0b[:], l0b_d.ap())
            nc.sync.dma_start(l1w[:], l1_d.ap())
            nc.sync.dma_start(l2w[:], l2_d.ap())
            nc.sync.dma_start(cst[:], cst_d.ap())

            b0c, b1c, b2c = cst[:, 0:1], cst[:, 1:2], cst[:, 2:3]
            g0c, g1c, g2c = cst[:, 3:4], cst[:, 4:5], cst[:, 5:6]
            t0c, t1c, t2c = cst[:, 6:7], cst[:, 7:8], cst[:, 8:9]
            zc = cst[:, 9:10]

            # per-chunk accumulator slots
            dbg = resid.tile([128, 64], dt.float32)
            nc.vector.memset(dbg[:], 0.0)
            sumA = resid.tile([128, n_slots], dt.float32)
            ssA = resid.tile([128, n_slots], dt.float32)
            sx1s = resid.tile([128, n_slots], dt.float32)
            nc.vector.memset(sx1s[:], 0.0)
            ss1s = resid.tile([128, n_slots], dt.float32)
            sx2s = resid.tile([128, n_slots], dt.float32)
            ss2as = resid.tile([128, n_slots], dt.float32)
            ss2bs = resid.tile([128, n_slots], dt.float32)

            xg3 = xg[:].rearrange("p (u d) -> p u d", d=1)

            # ================= phase A: gather + L0 + stats0 =================
            gtiles = {}
            for g_off, g_sz, g_col in gcs:
                lic = gpool.tile([128, GC // 16], dt.int16, tag="li")
                nc.sync.dma_start(lic[:, :g_sz // 16],
                                  lidx_d.ap()[:, g_col:g_col + g_sz // 16])
                g = gpool.tile([128, GC], dt.float32, tag="g")
                nc.gpsimd.ap_gather(
                    g[:, :g_sz].rearrange("p (n d) -> p n d", d=1),
                    xg3,
                    lic[:, :g_sz // 16],
                    channels=128, num_elems=u_cap, d=1, num_idxs=g_sz,
                )
                gv0 = g_off // KN
                gverts = g_sz // KN
                uc = gpool.tile([128, GC // KN], dt.float32, tag="uc")
                for uo in range(0, gverts, MC):
                    uw = min(MC, gverts - uo)
                    pu = psp.tile([128, MC], dt.float32, tag="pst")
                    nc.tensor.matmul(pu[:, :uw], l0a[:],
                                     xg[:, gv0 + uo:gv0 + uo + uw],
                                     start=True, stop=True)
                    nc.vector.tensor_copy(uc[:, uo:uo + uw], pu[:, :uw])
                gtiles[g_off] = (g, g_off, uc)

            for mo, w, ci in mcs:
                # find containing gather tile
                g, g_base, uc = None, None, None
                for g_off, g_sz, _ in gcs:
                    if g_off <= mo < g_off + g_sz:
                        g, g_base, uc = gtiles[g_off][0], g_off, gtiles[g_off][2]
                        break
                nv = w // KN
                ps = psp.tile([128, MC], dt.float32, tag="p1")
                nc.tensor.matmul(ps[:, :w], l0b[:], g[:, mo - g_base:mo - g_base + w],
                                 start=True, stop=True)
                uv = (mo - g_base) // KN
                ub = uc[:, uv:uv + nv].unsqueeze(2).broadcast_to((128, nv, KN))
                sl3 = y0[:, mo:mo + w].rearrange("p (n k) -> p n k", k=KN)
                nc.vector.scalar_tensor_tensor(
                    out=sl3,
                    in0=ps[:, :w].rearrange("p (n k) -> p n k", k=KN),
                    scalar=1.0, in1=ub, op0=OP.mult, op1=OP.add,
                    accum_out=sumA[:, ci:ci + 1])
                sl = y0[:, mo:mo + w]
                sq = scr.tile([128, MC], dt.float32, tag="sq")
                nc.scalar.activation(sq[:, :w], sl, AF.Square,
                                     accum_out=ssA[:, ci:ci + 1])

            # ---- stats0: reduce, allreduce, normalize consts ----
            def stats_allreduce(s_slot, q_slot, via_mm=None, extra_q=None,
                                swap_halves=True):
                loc = tiny.tile([128, 2], dt.float32, name="loc", uniquify=True)
                if via_mm is None:
                    nc.vector.tensor_reduce(loc[:, 0:1], s_slot[:],
                                            axis=mybir.AxisListType.X, op=OP.add)
                else:
                    nc.scalar.copy(loc[:, 0:1], via_mm[:])
                if extra_q is None:
                    nc.vector.tensor_reduce(loc[:, 1:2], q_slot[:],
                                            axis=mybir.AxisListType.X, op=OP.add)
                else:
                    qa = tiny.tile([128, 1], dt.float32, name="qa", uniquify=True)
                    qb = tiny.tile([128, 1], dt.float32, name="qb", uniquify=True)
                    nc.vector.tensor_reduce(qa[:], q_slot[:],
                                            axis=mybir.AxisListType.X, op=OP.add)
                    nc.vector.tensor_reduce(qb[:], extra_q[:],
                                            axis=mybir.AxisListType.X, op=OP.add)
                    nc.vector.tensor_tensor(loc[:, 1:2], qa[:], qb[:], op=OP.add)
                ib = drp.tile([128, 2], dt.float32, name="ib", uniquify=True)
                ob = drp.tile([128, 2], dt.float32, name="ob", uniquify=True)
                nc.sync.dma_start(ib[:], loc[:])
                if collectives:
                    nc.gpsimd.collective_compute(
                        "AllReduce", OP.add, replica_groups=rg,
                        ins=[ib[:].opt()], outs=[ob[:].opt()])
                else:
                    nc.sync.dma_start(ob[:], ib[:])
                sg = tiny.tile([128, 2], dt.float32, name="sg", uniquify=True)
                nc.sync.dma_start(sg[:], ob[:])
                tot = tiny.tile([128, 2], dt.float32, name="tot", uniquify=True)
                if swap_halves:
                    tmp = tiny.tile([128, 2], dt.float32, name="tmp", uniquify=True)
                    nc.sync.dma_start(tmp[0:64, :], sg[64:128, :])
                    nc.sync.dma_start(tmp[64:128, :], sg[0:64, :])
                    nc.vector.tensor_tensor(tot[:], sg[:], tmp[:], op=OP.add)
                else:
                    nc.vector.tensor_copy(tot[:], sg[:])
                return tot

            def bn_consts(tot, gch, bch, betach):
                mu = tiny.tile([128, 1], dt.float32, name="mu", uniquify=True)
                nc.vector.tensor_scalar_mul(mu[:], tot[:, 0:1], 1.0 / cnt)
                ex2 = tiny.tile([128, 1], dt.float32, name="ex2", uniquify=True)
                nc.vector.tensor_scalar_mul(ex2[:], tot[:, 1:2], 1.0 / cnt)
                var = tiny.tile([128, 1], dt.float32, name="var", uniquify=True)
                nc.vector.tensor_tensor(var[:], mu[:], mu[:], op=OP.mult)
                nc.vector.tensor_tensor(var[:], ex2[:], var[:], op=OP.subtract)
                nc.vector.tensor_scalar_add(var[:], var[:], EPS)
                inv = tiny.tile([128, 1], dt.float32, name="inv", uniquify=True)
                nc.vector.reciprocal(inv[:], var[:])
                s_ = tiny.tile([128, 1], dt.float32, name="s_", uniquify=True)
                nc.scalar.activation(s_[:], inv[:], AF.Sqrt)
                scl = tiny.tile([128, 1], dt.float32, name="scl", uniquify=True)
                nc.vector.tensor_tensor(scl[:], s_[:], gch, op=OP.mult)
                bia = tiny.tile([128, 1], dt.float32, name="bia", uniquify=True)
                nc.vector.tensor_tensor(bia[:], bch, mu[:], op=OP.subtract)
                nc.vector.tensor_tensor(bia[:], bia[:], scl[:], op=OP.mult)
                nc.vector.tensor_tensor(bia[:], bia[:], betach, op=OP.add)
                return scl, bia

            # NOTE: conv bias b_l is absorbed by the BN mean (zc = zeros col)
            tot0 = stats_allreduce(sumA, ssA)
            scl0, bia0 = bn_consts(tot0, g0c, zc, t0c)
            nc.vector.tensor_copy(dbg[:, 0:2], tot0[:])
            nc.vector.tensor_copy(dbg[:, 6:7], scl0[:])
            nc.vector.tensor_copy(dbg[:, 7:8], bia0[:])
            nc.vector.tensor_copy(dbg[:, 12:28], y0[:, 0:16])

            # ================= phase B: x1 = relu-affine(y0); L1 stats ========
            po = 0
            pj = 0
            while po < f_tot:
                pw = min(2 * MC, f_tot - po)
                sl = y0[:, po:po + pw]
                nc.scalar.activation(sl, sl, AF.Relu, bias=bia0, scale=scl0,
                                     accum_out=sx1s[:, pj:pj + 1])
                po += pw
                pj += 1
            for mo, w, ci in mcs:
                sl = y0[:, mo:mo + w]
                ps = psp.tile([128, MC], dt.float32, tag="p1")
                nc.tensor.matmul(ps[:, :w], l1w[:], sl, start=True, stop=True)
                sq = scr.tile([128, MC], dt.float32, tag="sq")
                nc.scalar.activation(sq[:, :w], ps[:, :w], AF.Square,
                                     accum_out=ss1s[:, ci:ci + 1])

            sx1r = tiny.tile([128, 1], dt.float32)
            nc.vector.tensor_reduce(sx1r[:], sx1s[:], axis=mybir.AxisListType.X,
                                    op=OP.add)
            ps1s = psp.tile([128, 1], dt.float32, tag="pst")
            nc.tensor.matmul(ps1s[:], l1w[:], sx1r[:], start=True, stop=True)
            tot1 = stats_allreduce(None, ss1s, via_mm=ps1s)
            scl1, bia1 = bn_consts(tot1, g1c, zc, t1c)
            nc.vector.tensor_copy(dbg[:, 2:4], tot1[:])
            nc.vector.tensor_copy(dbg[:, 8:9], scl1[:])
            nc.vector.tensor_copy(dbg[:, 9:10], bia1[:])
            nc.vector.tensor_copy(dbg[:, 44:45], sx1r[:])

            # ================= phase C: recompute L1, L2, stats2, k-max =======
            m2 = bigp.tile([128, 2, ns], dt.float32, tag="big")
            for mo, w, ci in mcs:
                nv = w // KN
                v0 = mo // KN
                ps1 = psp.tile([128, MC], dt.float32, tag="p1")
                nc.tensor.matmul(ps1[:, :w], l1w[:], y0[:, mo:mo + w],
                                 start=True, stop=True)
                x2 = x2p.tile([128, MC], dt.float32, tag="x2")
                nc.scalar.activation(x2[:, :w], ps1[:, :w], AF.Relu,
                                     bias=bia1, scale=scl1,
                                     accum_out=sx2s[:, ci:ci + 1])
                ps2a = psp.tile([128, MC], dt.float32, tag="p2a")
                ps2b = psp.tile([128, MC], dt.float32, tag="p2b")
                nc.tensor.matmul(ps2a[:, :w], l2w[0:64, :], x2[0:64, :w],
                                 start=True, stop=True, tile_position=(0, 0))
                nc.tensor.matmul(ps2b[:, :w], l2w[64:128, :], x2[64:128, :w],
                                 start=True, stop=True, tile_position=(64, 0))
                sqa = scr.tile([128, MC], dt.float32, tag="sq")
                nc.scalar.activation(sqa[:, :w], ps2a[:, :w], AF.Square,
                                     accum_out=ss2as[:, ci:ci + 1])
                sqb = scr.tile([128, MC], dt.float32, tag="sq")
                nc.scalar.activation(sqb[:, :w], ps2b[:, :w], AF.Square,
                                     accum_out=ss2bs[:, ci:ci + 1])
                nc.vector.tensor_reduce(
                    m2[:, 0, v0:v0 + nv],
                    ps2a[:, :w].rearrange("p (n k) -> p n k", k=KN),
                    axis=mybir.AxisListType.X, op=OP.max)
                nc.vector.tensor_reduce(
                    m2[:, 1, v0:v0 + nv],
                    ps2b[:, :w].rearrange("p (n k) -> p n k", k=KN),
                    axis=mybir.AxisListType.X, op=OP.max)

            sx2r = tiny.tile([128, 1], dt.float32)
            nc.vector.tensor_reduce(sx2r[:], sx2s[:], axis=mybir.AxisListType.X,
                                    op=OP.add)
            ps2sa = psp.tile([128, 1], dt.float32, tag="pst")
            ps2sb = psp.tile([128, 1], dt.float32, tag="pst")
            nc.tensor.matmul(ps2sa[:], l2w[0:64, :], sx2r[0:64, :], start=True, stop=True)
            nc.tensor.matmul(ps2sb[:], l2w[64:128, :], sx2r[64:128, :], start=True, stop=True)
            ps2s = tiny.tile([128, 1], dt.float32)
            nc.scalar.copy(ps2s[:], ps2sa[:])
            nc.vector.tensor_tensor(ps2s[:], ps2s[:], ps2sb[:], op=OP.add)
            tot2 = stats_allreduce(None, ss2as, via_mm=ps2s, extra_q=ss2bs,
                                   swap_halves=False)
            scl2, bia2 = bn_consts(tot2, g2c, zc, t2c)
            nc.vector.tensor_copy(dbg[:, 4:6], tot2[:])
            nc.vector.tensor_copy(dbg[:, 10:11], scl2[:])
            nc.vector.tensor_copy(dbg[:, 11:12], bia2[:])
            nc.vector.tensor_copy(dbg[:, 28:44], m2[:, 0, 0:16])
            nc.vector.tensor_copy(dbg[:, 45:46], sx2r[:])

            # ================= phase D: finalize =============================
            for b in range(2):
                nc.scalar.activation(m2[:, b, :], m2[:, b, :], AF.Relu,
                                     bias=bia2, scale=scl2)
            nc.sync.dma_start(newf_d.ap(), m2[:])
            PRC = 176
            for b in range(2):
                for c0 in range(0, ns, PRC):
                    cw = min(PRC, ns - c0)
                    pr = prp.tile([128, 176], dt.float32, tag="pr")
                    nc.gpsimd.partition_all_reduce(
                        pr[:, :cw], m2[:, b, c0:c0 + cw], channels=128,
                        reduce_op=bass_isa.ReduceOp.max)
                    nc.sync.dma_start(outf_d.ap()[b:b + 1, c0:c0 + cw],
                                      pr[0:1, :cw])
            nc.sync.dma_start(dbg_d.ap(), dbg[:])

    nc.compile()
    return nc


# ----------------------------------------------------------------------------
# host side
# ----------------------------------------------------------------------------

def _prep_core(i, xp, near_idx, u_cap=U_CAP, ns=NS, n_tot=N):
    """Build per-core gather table + local int16 indices (k-inner order)."""
    lo, hi = i * ns, (i + 1) * ns
    flat = near_idx[lo:hi].reshape(-1).astype(np.int64)       # [ns*KN], k-inner
    real = flat < n_tot
    own = np.arange(lo, hi, dtype=np.int64)
    uniq = np.unique(flat[real])
    ext = uniq[(uniq < lo) | (uniq >= hi)]
    order = np.concatenate([own, ext])
    n_ord = len(order)
    pad_idx = n_ord
    u = n_ord + 1
    assert u <= u_cap, f"unique count {u} exceeds U_CAP {u_cap}"
    lut = np.full(n_tot, -1, np.int64)
    lut[order] = np.arange(n_ord)
    lidx_flat = np.where(real, lut[np.clip(flat, 0, n_tot - 1)], pad_idx)
    assert (lidx_flat >= 0).all() and (lidx_flat < u).all()

    xg = np.zeros((128, u_cap), np.float32)
    xg[:, :n_ord] = xp[:, order]
    xg[:, pad_idx] = -1.0

    cols = []
    for off, sz, _ in _gather_chunks(ns * KN, GC):
        arr = lidx_flat[off:off + sz]
        cols.append(arr.reshape(sz // 16, 16).T)              # [16, sz/16]
    wrapped = np.concatenate(cols, axis=1).astype(np.int16)   # [16, idx_cols]
    lidx = np.tile(wrapped, (8, 1))                           # [128, idx_cols]
    return {"xg": xg, "lidx": lidx}


def _prep_shared(W0, b0, g0, beta0, W1, b1, g1, beta1, W2, b2, g2, beta2):
    def blockdiag(wT):
        z = np.zeros((128, 128), np.float32)
        z[0:64, 0:64] = wT
        z[64:128, 64:128] = wT
        return z

    l0a = blockdiag(np.ascontiguousarray(W0[:, :64].T))
    l0b = blockdiag(np.ascontiguousarray(W0[:, 64:].T))
    l1 = blockdiag(np.ascontiguousarray(W1.T))
    l2 = np.vstack([W2.T, W2.T]).astype(np.float32)           # [128, 128]
    p = np.arange(128)
    cst = np.zeros((128, 12), np.float32)
    cst[:, 0] = b0[p % 64]
    cst[:, 1] = b1[p % 64]
    cst[:, 2] = b2[p]
    cst[:, 3] = g0[p % 64]
    cst[:, 4] = g1[p % 64]
    cst[:, 5] = g2[p]
    cst[:, 6] = beta0[p % 64]
    cst[:, 7] = beta1[p % 64]
    cst[:, 8] = beta2[p]
    return {"l0a": l0a, "l0b": l0b, "l1": l1, "l2": l2, "cst": cst}


_COMPILED = None
_COMPILED_UCAP = None


def kernel(points, near_idx, W0, b0, g0, beta0, W1, b1, g1, beta1,
           W2, b2, g2, beta2):
    global _COMPILED
    import concourse.bass_utils as bass_utils

    points = np.asarray(points, np.float32)
    near_idx = np.asarray(near_idx)
    assert np.asarray(g2, np.float32).min() > 0, "layer-2 gamma must be >0 for k-max fold"

    xp = points.reshape(128, N)                               # [(b,c), n]
    shared = _prep_shared(W0, b0, g0, beta0, W1, b1, g1, beta1,
                          W2, b2, g2, beta2)
    shared = {k: np.asarray(v, np.float32) for k, v in shared.items()}
    in_maps = []
    for i in range(NCORES):
        m = dict(shared)
        m.update(_prep_core(i, xp, near_idx))
        in_maps.append(m)

    if _COMPILED is None:
        _COMPILED = _build()
    nc = _COMPILED

    import os
    res = bass_utils.run_bass_kernel_spmd(
        nc, in_maps, core_ids=list(range(NCORES)),
        trace=bool(os.environ.get("KERNEL_TRACE")))
    global LAST_RESULT
    LAST_RESULT = res

    new_feature = np.zeros((B, 128, N), np.float32)
    out_feature = np.zeros((B, N), np.float32)
    for i in range(NCORES):
        r = res.results[i]
        lo, hi = i * NS, (i + 1) * NS
        for b in range(B):
            new_feature[b, :, lo:hi] = r["newf"][:, b, :]
            out_feature[b, lo:hi] = r["outf"][b]
    return (out_feature, new_feature)


# revision 51
# speedup vs baseline: 1.0069x; 1.0069x over previous
"""Trainium2 Bass kernel for nn_EncodingLayer (gnn message passing).

Math (matches reference.py):
  x = points^T [B,N,C]; xpad = cat([x, -1]); near = xpad[:, near_idx]      (gather)
  feat = cat([x bcast over k, near])  -> 3x (1x1 conv + BN(train stats) + relu)
  new_feature = max over 6 neighbors; out_feature = max over channels.

Strategy: shard N over 8 cores.  Per core, the gather is done from an SBUF
table of this core's *unique* neighbor vertices (host computes the unique set
and remaps indices to local int16; table width < 32768 so GPSIMD ap_gather
applies, full fp32).  Layer l BN stats (sum, sumsq per channel) are
accumulated on-chip and combined with one 8-core AllReduce per layer; the
BN+relu collapses into the scalar-engine activation (per-partition scale and
bias).  Layer-1 activations are recomputed in phase C instead of stored so
everything stays in SBUF.  Neighbor-max commutes with the (monotone) final
affine+relu, so it runs on pre-BN y2 chunks straight out of PSUM.
"""

import numpy as np

B, C, N, KN = 2, 64, 40000, 6
NCORES = 8
NS = N // NCORES          # vertices per core
F = NS * KN               # free-size of per-core row axis (k-inner)
U_CAP = 16384             # compile-time gather-table width (>= per-core unique count)
CNT = float(B * N * KN)   # BN normalization count
EPS = 1e-5
GC = 3360                 # gather chunk (columns)
MC = 480                  # matmul/psum chunk (80 vertices x 6)


def _gather_chunks(f_tot, gc):
    out = []
    off = 0
    col = 0
    while off < f_tot:
        sz = min(gc, f_tot - off)
        assert sz % 16 == 0 and sz % KN == 0
        out.append((off, sz, col))
        col += sz // 16
        off += sz
    return out


def _mm_chunks(f_tot, gc, mc):
    out = []
    ci = 0
    for g_off, g_sz, _ in _gather_chunks(f_tot, gc):
        o = 0
        while o < g_sz:
            w = min(mc, g_sz - o)
            assert w % KN == 0
            out.append((g_off + o, w, ci))
            ci += 1
            o += w
    return out


def _build(u_cap=U_CAP, f_tot=F, ns=NS, idx_cols=None, n_slots=None, cnt=CNT, collectives=True, n_dev=NCORES):
    import concourse.bass as bass
    import concourse.mybir as mybir
    import concourse.bacc as bacc
    import concourse.tile as tile
    import concourse.bass_isa as bass_isa

    gcs = _gather_chunks(f_tot, GC)
    mcs = _mm_chunks(f_tot, GC, MC)
    if idx_cols is None:
        idx_cols = gcs[-1][2] + gcs[-1][1] // 16
    if n_slots is None:
        n_slots = len(mcs)

    dt = mybir.dt
    AF = mybir.ActivationFunctionType
    OP = mybir.AluOpType

    nc = bacc.Bacc("TRN2", target_bir_lowering=False, debug=False,
                   num_devices=n_dev)

    # ---- kernel I/O ----
    xg_d = nc.dram_tensor("xg", [128, u_cap], dt.float32, kind="ExternalInput")
    lidx_d = nc.dram_tensor("lidx", [128, idx_cols], dt.int16, kind="ExternalInput")
    l0a_d = nc.dram_tensor("l0a", [128, 128], dt.float32, kind="ExternalInput")
    l0b_d = nc.dram_tensor("l0b", [128, 128], dt.float32, kind="ExternalInput")
    l1_d = nc.dram_tensor("l1", [128, 128], dt.float32, kind="ExternalInput")
    l2_d = nc.dram_tensor("l2", [128, 128], dt.float32, kind="ExternalInput")
    cst_d = nc.dram_tensor("cst", [128, 12], dt.float32, kind="ExternalInput")
    newf_d = nc.dram_tensor("newf", [128, 2, ns], dt.float32, kind="ExternalOutput")
    outf_d = nc.dram_tensor("outf", [2, ns], dt.float32, kind="ExternalOutput")
    dbg_d = nc.dram_tensor("dbg", [128, 64], dt.float32, kind="ExternalOutput")

    rg = [list(range(NCORES))]

    with tile.TileContext(nc) as tc:
        with (
            tc.tile_pool(name="bigp", bufs=1) as bigp,
            tc.tile_pool(name="resid", bufs=1) as resid,
            tc.tile_pool(name="gpool", bufs=1) as gpool,
            tc.tile_pool(name="scr", bufs=1) as scr,
            tc.tile_pool(name="x2p", bufs=2) as x2p,
            tc.tile_pool(name="tiny", bufs=1) as tiny,
            tc.tile_pool(name="prp", bufs=1) as prp,
            tc.tile_pool(name="psum", bufs=2, space="PSUM") as psp,
            tc.tile_pool(name="dram", bufs=6, space="DRAM") as drp,
        ):
            # ---- resident tiles ----
            xg = bigp.tile([128, u_cap], dt.float32, tag="big")
            y0 = resid.tile([128, f_tot], dt.float32)
            l0a = resid.tile([128, 128], dt.float32)
            l0b = resid.tile([128, 128], dt.float32)
            l1w = resid.tile([128, 128], dt.float32)
            l2w = resid.tile([128, 128], dt.float32)
            cst = resid.tile([128, 12], dt.float32)

            nc.sync.dma_start(xg[:], xg_d.ap())
            nc.sync.dma_start(l0a[:], l0a_d.ap())
            nc.sync.dma_start(l0b[:], l0b_d.ap())
            nc.sync.dma_start(l1w[:], l1_d.ap())
            nc.sync.dma_start(l2w[:], l2_d.ap())
            nc.sync.dma_start(cst[:], cst_d.ap())

            b0c, b1c, b2c = cst[:, 0:1], cst[:, 1:2], cst[:, 2:3]
            g0c, g1c, g2c = cst[:, 3:4], cst[:, 4:5], cst[:, 5:6]
            t0c, t1c, t2c = cst[:, 6:7], cst[:, 7:8], cst[:, 8:9]
            zc = cst[:, 9:10]

            # per-chunk accumulator slots
            dbg = resid.tile([128, 64], dt.float32)
            nc.vector.memset(dbg[:], 0.0)
            sumA = resid.tile([128, n_slots], dt.float32)
            ssA = resid.tile([128, n_slots], dt.float32)
            sx1s = resid.tile([128, n_slots], dt.float32)
            nc.vector.memset(sx1s[:], 0.0)
            ss1s = resid.tile([128, n_slots], dt.float32)
            sx2s = resid.tile([128, n_slots], dt.float32)
            ss2as = resid.tile([128, n_slots], dt.float32)
            ss2bs = resid.tile([128, n_slots], dt.float32)

            xg3 = xg[:].rearrange("p (u d) -> p u d", d=1)

            # ================= phase A: gather + L0 + stats0 =================
            gtiles = {}
            for g_off, g_sz, g_col in gcs:
                lic = gpool.tile([128, GC // 16], dt.int16, tag="li")
                nc.sync.dma_start(lic[:, :g_sz // 16],
                                  lidx_d.ap()[:, g_col:g_col + g_sz // 16])
                g = gpool.tile([128, GC], dt.float32, tag="g")
                nc.gpsimd.ap_gather(
                    g[:, :g_sz].rearrange("p (n d) -> p n d", d=1),
                    xg3,
                    lic[:, :g_sz // 16],
                    channels=128, num_elems=u_cap, d=1, num_idxs=g_sz,
                )
                gv0 = g_off // KN
                gverts = g_sz // KN
                uc = gpool.tile([128, GC // KN], dt.float32, tag="uc")
                for uo in range(0, gverts, MC):
                    uw = min(MC, gverts - uo)
                    pu = psp.tile([128, MC], dt.float32, tag="pst")
                    nc.tensor.matmul(pu[:, :uw], l0a[:],
                                     xg[:, gv0 + uo:gv0 + uo + uw],
                                     start=True, stop=True)
                    nc.vector.tensor_copy(uc[:, uo:uo + uw], pu[:, :uw])
                gtiles[g_off] = (g, g_off, uc)

            for mo, w, ci in mcs:
                # find containing gather tile
                g, g_base, uc = None, None, None
                for g_off, g_sz, _ in gcs:
                    if g_off <= mo < g_off + g_sz:
                        g, g_base, uc = gtiles[g_off][0], g_off, gtiles[g_off][2]
                        break
                nv = w // KN
                ps = psp.tile([128, MC], dt.float32, tag="p1")
                nc.tensor.matmul(ps[:, :w], l0b[:], g[:, mo - g_base:mo - g_base + w],
                                 start=True, stop=True)
                uv = (mo - g_base) // KN
                ub = uc[:, uv:uv + nv].unsqueeze(2).broadcast_to((128, nv, KN))
                sl3 = y0[:, mo:mo + w].rearrange("p (n k) -> p n k", k=KN)
                nc.vector.scalar_tensor_tensor(
                    out=sl3,
                    in0=ps[:, :w].rearrange("p (n k) -> p n k", k=KN),
                    scalar=1.0, in1=ub, op0=OP.mult, op1=OP.add,
                    accum_out=sumA[:, ci:ci + 1])
                sl = y0[:, mo:mo + w]
                sq = scr.tile([128, MC], dt.float32, tag="sq")
                nc.scalar.activation(sq[:, :w], sl, AF.Square,
                                     accum_out=ssA[:, ci:ci + 1])

            # ---- stats0: reduce, allreduce, normalize consts ----
            def stats_allreduce(s_slot, q_slot, via_mm=None, extra_q=None,
                                swap_halves=True):
                loc = tiny.tile([128, 2], dt.float32, name="loc", uniquify=True)
                if via_mm is None:
                    nc.vector.tensor_reduce(loc[:, 0:1], s_slot[:],
                                            axis=mybir.AxisListType.X, op=OP.add)
                else:
                    nc.scalar.copy(loc[:, 0:1], via_mm[:])
                if extra_q is None:
                    nc.vector.tensor_reduce(loc[:, 1:2], q_slot[:],
                                            axis=mybir.AxisListType.X, op=OP.add)
                else:
                    qa = tiny.tile([128, 1], dt.float32, name="qa", uniquify=True)
                    qb = tiny.tile([128, 1], dt.float32, name="qb", uniquify=True)
                    nc.vector.tensor_reduce(qa[:], q_slot[:],
                                            axis=mybir.AxisListType.X, op=OP.add)
                    nc.vector.tensor_reduce(qb[:], extra_q[:],
                                            axis=mybir.AxisListType.X, op=OP.add)
                    nc.vector.tensor_tensor(loc[:, 1:2], qa[:], qb[:], op=OP.add)
                ib = drp.tile([128, 2], dt.float32, name="ib", uniquify=True)
                ob = drp.tile([128, 2], dt.float32, name="ob", uniquify=True)
                nc.sync.dma_start(ib[:], loc[:])
                if collectives:
                    nc.gpsimd.collective_compute(
                        "AllReduce", OP.add, replica_groups=rg,
                        ins=[ib[:].opt()], outs=[ob[:].opt()])
                else:
                    nc.sync.dma_start(ob[:], ib[:])
                sg = tiny.tile([128, 2], dt.float32, name="sg", uniquify=True)
                nc.sync.dma_start(sg[:], ob[:])
                tot = tiny.tile([128, 2], dt.float32, name="tot", uniquify=True)
                if swap_halves:
                    tmp = tiny.tile([128, 2], dt.float32, name="tmp", uniquify=True)
                    nc.sync.dma_start(tmp[0:64, :], sg[64:128, :])
                    nc.sync.dma_start(tmp[64:128, :], sg[0:64, :])
                    nc.vector.tensor_tensor(tot[:], sg[:], tmp[:], op=OP.add)
                else:
                    nc.vector.tensor_copy(tot[:], sg[:])
                return tot

            def bn_consts(tot, gch, bch, betach):
                mu = tiny.tile([128, 1], dt.float32, name="mu", uniquify=True)
                nc.vector.tensor_scalar_mul(mu[:], tot[:, 0:1], 1.0 / cnt)
                ex2 = tiny.tile([128, 1], dt.float32, name="ex2", uniquify=True)
                nc.vector.tensor_scalar_mul(ex2[:], tot[:, 1:2], 1.0 / cnt)
                var = tiny.tile([128, 1], dt.float32, name="var", uniquify=True)
                nc.vector.tensor_tensor(var[:], mu[:], mu[:], op=OP.mult)
                nc.vector.tensor_tensor(var[:], ex2[:], var[:], op=OP.subtract)
                nc.vector.tensor_scalar_add(var[:], var[:], EPS)
                inv = tiny.tile([128, 1], dt.float32, name="inv", uniquify=True)
                nc.vector.reciprocal(inv[:], var[:])
                s_ = tiny.tile([128, 1], dt.float32, name="s_", uniquify=True)
                nc.scalar.activation(s_[:], inv[:], AF.Sqrt)
                scl = tiny.tile([128, 1], dt.float32, name="scl", uniquify=True)
                nc.vector.tensor_tensor(scl[:], s_[:], gch, op=OP.mult)
                bia = tiny.tile([128, 1], dt.float32, name="bia", uniquify=True)
                nc.vector.tensor_tensor(bia[:], bch, mu[:], op=OP.subtract)
                nc.vector.tensor_tensor(bia[:], bia[:], scl[:], op=OP.mult)
                nc.vector.tensor_tensor(bia[:], bia[:], betach, op=OP.add)
                return scl, bia

            # NOTE: conv bias b_l is absorbed by the BN mean (zc = zeros col)
            tot0 = stats_allreduce(sumA, ssA)
            scl0, bia0 = bn_consts(tot0, g0c, zc, t0c)
            nc.vector.tensor_copy(dbg[:, 0:2], tot0[:])
            nc.vector.tensor_copy(dbg[:, 6:7], scl0[:])
            nc.vector.tensor_copy(dbg[:, 7:8], bia0[:])
            nc.vector.tensor_copy(dbg[:, 12:28], y0[:, 0:16])

            # ================= phase B: x1 = relu-affine(y0); L1 stats ========
            po = 0
            pj = 0
            while po < f_tot:
                pw = min(2 * MC, f_tot - po)
                sl = y0[:, po:po + pw]
                nc.scalar.activation(sl, sl, AF.Relu, bias=bia0, scale=scl0,
                                     accum_out=sx1s[:, pj:pj + 1])
                po += pw
                pj += 1
            for mo, w, ci in mcs:
                sl = y0[:, mo:mo + w]
                ps = psp.tile([128, MC], dt.float32, tag="p1")
                nc.tensor.matmul(ps[:, :w], l1w[:], sl, start=True, stop=True)
                sq = scr.tile([128, MC], dt.float32, tag="sq")
                nc.scalar.activation(sq[:, :w], ps[:, :w], AF.Square,
                                     accum_out=ss1s[:, ci:ci + 1])

            sx1r = tiny.tile([128, 1], dt.float32)
            nc.vector.tensor_reduce(sx1r[:], sx1s[:], axis=mybir.AxisListType.X,
                                    op=OP.add)
            ps1s = psp.tile([128, 1], dt.float32, tag="pst")
            nc.tensor.matmul(ps1s[:], l1w[:], sx1r[:], start=True, stop=True)
            tot1 = stats_allreduce(None, ss1s, via_mm=ps1s)
            scl1, bia1 = bn_consts(tot1, g1c, zc, t1c)
            nc.vector.tensor_copy(dbg[:, 2:4], tot1[:])
            nc.vector.tensor_copy(dbg[:, 8:9], scl1[:])
            nc.vector.tensor_copy(dbg[:, 9:10], bia1[:])
            nc.vector.tensor_copy(dbg[:, 44:45], sx1r[:])

            # ================= phase C: recompute L1, L2, stats2, k-max =======
            m2 = bigp.tile([128, 2, ns], dt.float32, tag="big")
            for mo, w, ci in mcs:
                nv = w // KN
                v0 = mo // KN
                ps1 = psp.tile([128, MC], dt.float32, tag="p1")
                nc.tensor.matmul(ps1[:, :w], l1w[:], y0[:, mo:mo + w],
                                 start=True, stop=True)
                x2 = x2p.tile([128, MC], dt.float32, tag="x2")
                nc.scalar.activation(x2[:, :w], ps1[:, :w], AF.Relu,
                                     bias=bia1, scale=scl1,
                                     accum_out=sx2s[:, ci:ci + 1])
                ps2 = psp.tile([128, 2, 512], dt.float32, tag="p2a")
                nc.tensor.matmul(ps2[:, 0, :w], l2w[0:64, :], x2[0:64, :w],
                                 start=True, stop=True, tile_position=(0, 0))
                nc.tensor.matmul(ps2[:, 1, :w], l2w[64:128, :], x2[64:128, :w],
                                 start=True, stop=True, tile_position=(64, 0))
                nc.vector.tensor_reduce(
                    m2[:, 0, v0:v0 + nv],
                    ps2[:, 0, :w].rearrange("p (n k) -> p n k", k=KN),
                    axis=mybir.AxisListType.X, op=OP.max)
                nc.vector.tensor_reduce(
                    m2[:, 1, v0:v0 + nv],
                    ps2[:, 1, :w].rearrange("p (n k) -> p n k", k=KN),
                    axis=mybir.AxisListType.X, op=OP.max)
                nc.scalar.activation(ps2[:, :, :w], ps2[:, :, :w], AF.Square,
                                     accum_out=ss2as[:, ci:ci + 1])

            sx2r = tiny.tile([128, 1], dt.float32)
            nc.vector.tensor_reduce(sx2r[:], sx2s[:], axis=mybir.AxisListType.X,
                                    op=OP.add)
            ps2sa = psp.tile([128, 1], dt.float32, tag="pst")
            ps2sb = psp.tile([128, 1], dt.float32, tag="pst")
            nc.tensor.matmul(ps2sa[:], l2w[0:64, :], sx2r[0:64, :], start=True, stop=True)
            nc.tensor.matmul(ps2sb[:], l2w[64:128, :], sx2r[64:128, :], start=True, stop=True)
            ps2s = tiny.tile([128, 1], dt.float32)
            nc.scalar.copy(ps2s[:], ps2sa[:])
            nc.vector.tensor_tensor(ps2s[:], ps2s[:], ps2sb[:], op=OP.add)
            tot2 = stats_allreduce(None, ss2as, via_mm=ps2s,
                                   swap_halves=False)
            scl2, bia2 = bn_consts(tot2, g2c, zc, t2c)
            nc.vector.tensor_copy(dbg[:, 4:6], tot2[:])
            nc.vector.tensor_copy(dbg[:, 10:11], scl2[:])
            nc.vector.tensor_copy(dbg[:, 11:12], bia2[:])
            nc.vector.tensor_copy(dbg[:, 28:44], m2[:, 0, 0:16])
            nc.vector.tensor_copy(dbg[:, 45:46], sx2r[:])

            # ================= phase D: finalize =============================
            for b in range(2):
                nc.scalar.activation(m2[:, b, :], m2[:, b, :], AF.Relu,
                                     bias=bia2, scale=scl2)
            nc.sync.dma_start(newf_d.ap(), m2[:])
            PRC = 176
            for b in range(2):
                for c0 in range(0, ns, PRC):
                    cw = min(PRC, ns - c0)
                    pr = prp.tile([128, 176], dt.float32, tag="pr")
                    nc.gpsimd.partition_all_reduce(
                        pr[:, :cw], m2[:, b, c0:c0 + cw], channels=128,
                        reduce_op=bass_isa.ReduceOp.max)
                    nc.sync.dma_start(outf_d.ap()[b:b + 1, c0:c0 + cw],
                                      pr[0:1, :cw])
            nc.sync.dma_start(dbg_d.ap(), dbg[:])

    nc.compile()
    return nc


# ----------------------------------------------------------------------------
# host side
# ----------------------------------------------------------------------------

def _prep_core(i, xp, near_idx, u_cap=U_CAP, ns=NS, n_tot=N):
    """Build per-core gather table + local int16 indices (k-inner order)."""
    lo, hi = i * ns, (i + 1) * ns
    flat = near_idx[lo:hi].reshape(-1).astype(np.int64)       # [ns*KN], k-inner
    real = flat < n_tot
    own = np.arange(lo, hi, dtype=np.int64)
    uniq = np.unique(flat[real])
    ext = uniq[(uniq < lo) | (uniq >= hi)]
    order = np.concatenate([own, ext])
    n_ord = len(order)
    pad_idx = n_ord
    u = n_ord + 1
    assert u <= u_cap, f"unique count {u} exceeds U_CAP {u_cap}"
    lut = np.full(n_tot, -1, np.int64)
    lut[order] = np.arange(n_ord)
    lidx_flat = np.where(real, lut[np.clip(flat, 0, n_tot - 1)], pad_idx)
    assert (lidx_flat >= 0).all() and (lidx_flat < u).all()

    xg = np.zeros((128, u_cap), np.float32)
    xg[:, :n_ord] = xp[:, order]
    xg[:, pad_idx] = -1.0

    cols = []
    for off, sz, _ in _gather_chunks(ns * KN, GC):
        arr = lidx_flat[off:off + sz]
        cols.append(arr.reshape(sz // 16, 16).T)              # [16, sz/16]
    wrapped = np.concatenate(cols, axis=1).astype(np.int16)   # [16, idx_cols]
    lidx = np.tile(wrapped, (8, 1))                           # [128, idx_cols]
    return {"xg": xg, "lidx": lidx}


def _prep_shared(W0, b0, g0, beta0, W1, b1, g1, beta1, W2, b2, g2, beta2):
    def blockdiag(wT):
        z = np.zeros((128, 128), np.float32)
        z[0:64, 0:64] = wT
        z[64:128, 64:128] = wT
        return z

    l0a = blockdiag(np.ascontiguousarray(W0[:, :64].T))
    l0b = blockdiag(np.ascontiguousarray(W0[:, 64:].T))
    l1 = blockdiag(np.ascontiguousarray(W1.T))
    l2 = np.vstack([W2.T, W2.T]).astype(np.float32)           # [128, 128]
    p = np.arange(128)
    cst = np.zeros((128, 12), np.float32)
    cst[:, 0] = b0[p % 64]
    cst[:, 1] = b1[p % 64]
    cst[:, 2] = b2[p]
    cst[:, 3] = g0[p % 64]
    cst[:, 4] = g1[p % 64]
    cst[:, 5] = g2[p]
    cst[:, 6] = beta0[p % 64]
    cst[:, 7] = beta1[p % 64]
    cst[:, 8] = beta2[p]
    return {"l0a": l0a, "l0b": l0b, "l1": l1, "l2": l2, "cst": cst}


_COMPILED = None
_COMPILED_UCAP = None


def kernel(points, near_idx, W0, b0, g0, beta0, W1, b1, g1, beta1,
           W2, b2, g2, beta2):
    global _COMPILED
    import concourse.bass_utils as bass_utils

    points = np.asarray(points, np.float32)
    near_idx = np.asarray(near_idx)
    assert np.asarray(g2, np.float32).min() > 0, "layer-2 gamma must be >0 for k-max fold"

    xp = points.reshape(128, N)                               # [(b,c), n]
    shared = _prep_shared(W0, b0, g0, beta0, W1, b1, g1, beta1,
                          W2, b2, g2, beta2)
    shared = {k: np.asarray(v, np.float32) for k, v in shared.items()}
    in_maps = []
    for i in range(NCORES):
        m = dict(shared)
        m.update(_prep_core(i, xp, near_idx))
        in_maps.append(m)

    if _COMPILED is None:
        _COMPILED = _build()
    nc = _COMPILED

    import os
    res = bass_utils.run_bass_kernel_spmd(
        nc, in_maps, core_ids=list(range(NCORES)),
        trace=bool(os.environ.get("KERNEL_TRACE")))
    global LAST_RESULT
    LAST_RESULT = res

    new_feature = np.zeros((B, 128, N), np.float32)
    out_feature = np.zeros((B, N), np.float32)
    for i in range(NCORES):
        r = res.results[i]
        lo, hi = i * NS, (i + 1) * NS
        for b in range(B):
            new_feature[b, :, lo:hi] = r["newf"][:, b, :]
            out_feature[b, lo:hi] = r["outf"][b]
    return (out_feature, new_feature)
